# revision 37
# baseline (speedup 1.0000x reference)
"""Bag-of-words histogram kernel for Trainium2 (Bass/Tile), 8-core data-parallel.

Problem: docs [256, 2048] int32 token ids in [0, 32000) ->
         hist [256, 32000] fp32, hist[b, v] = count(docs[b, :] == v) / 2048.

Algorithm (per core, 32 rows):
  Factor each token t = 256*hi + lo (hi < 125, lo < 256). Then
    hist[b, hi, lo] = sum_s onehot_hi[s, hi] * onehot_lo[s, lo]
  accumulated over 16 K-tiles per row on the PE.

  One-hots are built with big DVE tensor_tensor is_equal ops (bf16, 2x DVE
  mode) covering TWO rows each, k-tile axis innermost (stride 1) so every
  operand's last AP dim is contiguous. The lo side uses quad-k tiles so the
  matmul rhs has an 8-byte column stride (full-speed PE streaming; 4B/32B
  cost ~2x). Weights (lhsT) tolerate strides. Row pairs in ACT_ROWS build
  their lo one-hots on the ACT engine instead (Square+Relu hat function),
  balancing the bottleneck DVE engine against otherwise-idle ACT cycles.
  All group DMAs + int preprocessing issue upfront so DVE never stalls on
  input loads.

Sharding: batch axis split 8 ways (32 rows per core), no communication.
"""

import sys

import numpy as np

for _p in ("/opt/trn_rl_repo",):
    if _p not in sys.path:
        sys.path.append(_p)

BATCH = 256
SEQ = 2048
VOCAB = 32000
N_CORES = 8
ROWS = BATCH // N_CORES  # 32 rows per core
P = 128
KT = SEQ // P            # 16 K-tiles per row
KQ = 4                   # k-tiles per lo quad tile (8B matmul rhs stride)
GR = 8                   # rows per input-DMA group
NLO = 256                # low-digit bins (t & 255)
NHI = 128                # high-digit compare width (t >> 8 < 125)

# Row pairs whose lo one-hots build on ACT (must be {even, even+1} pairs).
ACT_ROWS = frozenset({8, 9, 20, 21, 28, 29})


def _build_nc():
    from contextlib import ExitStack

    from concourse import bacc, bass, mybir
    from concourse.tile import TileContext

    nc = bacc.Bacc()
    docs = nc.dram_tensor("docs", [ROWS, SEQ], mybir.dt.int32, kind="ExternalInput")
    hist = nc.dram_tensor("hist", [ROWS, VOCAB], mybir.dt.float32, kind="ExternalOutput")

    f32 = mybir.dt.float32
    bf16 = mybir.dt.bfloat16
    Alu = mybir.AluOpType
    Act = mybir.ActivationFunctionType

    with TileContext(nc) as tc, ExitStack() as ctx:
        const_tp = ctx.enter_context(tc.tile_pool(name="const", bufs=1))
        tok_tp = ctx.enter_context(tc.tile_pool(name="tok", bufs=4))
        hilo_tp = ctx.enter_context(tc.tile_pool(name="hilo", bufs=4))
        ohh_tp = ctx.enter_context(tc.tile_pool(name="ohh", bufs=5))
        ohl_tp = ctx.enter_context(tc.tile_pool(name="ohl", bufs=3))
        act_tp = ctx.enter_context(tc.tile_pool(name="act", bufs=2))
        res_tp = ctx.enter_context(tc.tile_pool(name="res", bufs=4))
        psum_tp = ctx.enter_context(tc.tile_pool(name="psum", bufs=8, space="PSUM"))

        # Replicated iota constants, k-tile axis innermost (stride 1) so the
        # one-hot tensor_tensor ops keep contiguous last dims (2x DVE mode).
        iota_hi = const_tp.tile([P, NHI, KT], bf16)
        nc.gpsimd.iota(iota_hi[:], [[1, NHI], [0, KT]], channel_multiplier=0,
                       allow_small_or_imprecise_dtypes=True)
        iota_lo = const_tp.tile([P, NLO, KQ], bf16)
        nc.gpsimd.iota(iota_lo[:], [[1, NLO], [0, KQ]], channel_multiplier=0,
                       allow_small_or_imprecise_dtypes=True)
        iota_lof = const_tp.tile([P, NLO], bf16)  # flat, for the ACT path
        nc.gpsimd.iota(iota_lof[:], [[1, NLO]], channel_multiplier=0,
                       allow_small_or_imprecise_dtypes=True)

        # Upfront: all group DMAs + int preprocessing, so the one-hot DVE
        # stream never waits on input loads.
        hi_bfs, lo_bfs, nlo_fs = [], [], []
        for g in range(ROWS // GR):
            tok = tok_tp.tile([P, GR, KT], mybir.dt.int32, tag=f"t{g}")
            src = bass.AP(docs, g * GR * SEQ, [[16, P], [SEQ, GR], [1, KT]])
            nc.sync.dma_start(out=tok[:], in_=src)

            hi_i = hilo_tp.tile([P, GR, KT], mybir.dt.int32, tag=f"hii{g}")
            lo_i = hilo_tp.tile([P, GR, KT], mybir.dt.int32, tag=f"loi{g}")
            nc.vector.tensor_scalar(out=hi_i[:], in0=tok[:], scalar1=8,
                                    scalar2=None, op0=Alu.logical_shift_right)
            nc.vector.tensor_scalar(out=lo_i[:], in0=tok[:], scalar1=255,
                                    scalar2=None, op0=Alu.bitwise_and)
            hi_bf = hilo_tp.tile([P, GR, KT], bf16, tag=f"hib{g}")
            lo_bf = hilo_tp.tile([P, GR, KT], bf16, tag=f"lob{g}")
            nc.vector.tensor_scalar(out=hi_bf[:], in0=hi_i[:], scalar1=1.0,
                                    scalar2=None, op0=Alu.mult)
            nc.vector.tensor_scalar(out=lo_bf[:], in0=lo_i[:], scalar1=1.0,
                                    scalar2=None, op0=Alu.mult)
            nlo_f = None
            if any(g * GR <= r < (g + 1) * GR for r in ACT_ROWS):
                nlo_f = hilo_tp.tile([P, GR, KT], f32, tag=f"nlo{g}")
                nc.vector.tensor_scalar(out=nlo_f[:], in0=lo_i[:],
                                        scalar1=-1.0, scalar2=None,
                                        op0=Alu.mult)
            hi_bfs.append(hi_bf)
            lo_bfs.append(lo_bf)
            nlo_fs.append(nlo_f)

        def emit_matmuls(r_pair, oh_hi, lo_tiles):
            # The pair's two rows accumulate in two PSUM banks with their
            # k-streams interleaved, so consecutive matmuls never target the
            # same PSUM region (avoids any accumulate read-modify-write
            # bubble between back-to-back matmuls).
            ps0 = psum_tp.tile([P, NLO], f32, tag="ps")
            ps1 = psum_tp.tile([P, NLO], f32, tag="ps")
            pss = (ps0, ps1)
            for k in range(KT):
                for rr in range(2):
                    ent = lo_tiles[(rr, k)]
                    rhs = ent[0][:, ent[1], :, ent[2]] \
                        if isinstance(ent, tuple) else ent[:]
                    nc.tensor.matmul(out=pss[rr][:],
                                     lhsT=oh_hi[:, rr, :, k], rhs=rhs,
                                     start=(k == 0), stop=(k == KT - 1))
            for rr in range(2):
                res = res_tp.tile([P, NLO], f32)
                nc.scalar.mul(out=res[:], in_=pss[rr][:], mul=1.0 / SEQ)
                nc.sync.dma_start(
                    out=hist[r_pair + rr].rearrange("(h l) -> h l", l=NLO),
                    in_=res[:VOCAB // NLO, :])

        # ACT-pair matmuls are deferred two pair-slots so the PE keeps
        # streaming DVE pairs while the (slow) ACT builds accumulate;
        # otherwise the PE stalls in issue order, the one-hot rings fill,
        # and the bottleneck DVE engine starves (idle + wake-up penalty).
        pending = []
        for g in range(ROWS // GR):
            hi_bf, lo_bf, nlo_f = hi_bfs[g], lo_bfs[g], nlo_fs[g]
            for r0 in range(0, GR, 2):
                r_pair = g * GR + r0
                on_act = r_pair in ACT_ROWS
                # hi one-hots for 2 rows x 16 k-tiles in one op
                oh_hi = ohh_tp.tile([P, 2, NHI, KT], bf16)
                nc.vector.tensor_tensor(
                    out=oh_hi[:], in0=iota_hi[:].unsqueeze(1)
                        .broadcast_to([P, 2, NHI, KT]),
                    in1=hi_bf[:, r0:r0 + 2, :].unsqueeze(2)
                        .broadcast_to([P, 2, NHI, KT]),
                    op=Alu.is_equal)

                lo_tiles = {}
                if on_act:
                    # ACT path: per (row, k-tile), relu(1 - (iota - lo)^2)
                    for rr in range(2):
                        for k in range(KT):
                            sq = act_tp.tile([P, NLO], bf16, tag=f"sq{rr}")
                            t_lo = act_tp.tile([P, NLO], bf16,
                                               tag=f"a{rr}_{k}")
                            nc.scalar.activation(
                                out=sq[:], in_=iota_lof[:], func=Act.Square,
                                bias=nlo_f[:, r0 + rr, k:k + 1], scale=1.0)
                            nc.scalar.activation(
                                out=t_lo[:], in_=sq[:], func=Act.Relu,
                                bias=1.0, scale=-1.0)
                            lo_tiles[(rr, k)] = t_lo
                    pending.append((r_pair, oh_hi, lo_tiles))
                else:
                    # DVE path: 2-row quad-k tiles [p, 2, bin, 4]
                    for q in range(KT // KQ):
                        t_lo = ohl_tp.tile([P, 2, NLO, KQ], bf16, tag=f"q{q}")
                        nc.vector.tensor_tensor(
                            out=t_lo[:], in0=iota_lo[:].unsqueeze(1)
                                .broadcast_to([P, 2, NLO, KQ]),
                            in1=lo_bf[:, r0:r0 + 2, q * KQ:(q + 1) * KQ]
                                .unsqueeze(2).broadcast_to([P, 2, NLO, KQ]),
                            op=Alu.is_equal)
                        for rr in range(2):
                            for k in range(KQ):
                                lo_tiles[(rr, q * KQ + k)] = (t_lo, rr, k)

                    while pending and pending[0][0] <= r_pair - 4:
                        emit_matmuls(*pending.pop(0))
                    emit_matmuls(r_pair, oh_hi, lo_tiles)
        for args in pending:
            emit_matmuls(*args)
    nc.compile()
    return nc


_NC_CACHE = None


def _get_nc():
    global _NC_CACHE
    if _NC_CACHE is None:
        _NC_CACHE = _build_nc()
    return _NC_CACHE


def run_sharded(docs: np.ndarray, trace: bool = False):
    """Run the 8-core SPMD kernel. Returns (full_output, BassKernelResults)."""
    from concourse.bass_utils import run_bass_kernel_spmd

    docs = np.ascontiguousarray(np.asarray(docs, dtype=np.int32))
    assert docs.shape == (BATCH, SEQ), docs.shape
    shards = docs.reshape(N_CORES, ROWS, SEQ)
    in_maps = [{"docs": shards[i]} for i in range(N_CORES)]
    res = run_bass_kernel_spmd(_get_nc(), in_maps, core_ids=list(range(N_CORES)),
                               trace=trace)
    out = np.concatenate([res.results[i]["hist"] for i in range(N_CORES)], axis=0)
    return out, res


def kernel(docs: np.ndarray) -> np.ndarray:
    out, _ = run_sharded(docs, trace=False)
    return out


# revision 38
# speedup vs baseline: 1.0098x; 1.0098x over previous
"""Bag-of-words histogram kernel for Trainium2 (Bass/Tile), 8-core data-parallel.

Problem: docs [256, 2048] int32 token ids in [0, 32000) ->
         hist [256, 32000] fp32, hist[b, v] = count(docs[b, :] == v) / 2048.

Algorithm (per core, 32 rows):
  Factor each token t = 256*hi + lo (hi < 125, lo < 256). Then
    hist[b, hi, lo] = sum_s onehot_hi[s, hi] * onehot_lo[s, lo]
  accumulated over 16 K-tiles per row on the PE.

  One-hots are built with big DVE tensor_tensor is_equal ops (bf16, 2x DVE
  mode) covering TWO rows each, k-tile axis innermost (stride 1) so every
  operand's last AP dim is contiguous. The lo side uses quad-k tiles so the
  matmul rhs has an 8-byte column stride (full-speed PE streaming; 4B/32B
  cost ~2x). Weights (lhsT) tolerate strides. Row pairs in ACT_ROWS build
  their lo one-hots on the ACT engine instead (Square+Relu hat function),
  balancing the bottleneck DVE engine against otherwise-idle ACT cycles.
  All group DMAs + int preprocessing issue upfront so DVE never stalls on
  input loads.

Sharding: batch axis split 8 ways (32 rows per core), no communication.
"""

import sys

import numpy as np

for _p in ("/opt/trn_rl_repo",):
    if _p not in sys.path:
        sys.path.append(_p)

BATCH = 256
SEQ = 2048
VOCAB = 32000
N_CORES = 8
ROWS = BATCH // N_CORES  # 32 rows per core
P = 128
KT = SEQ // P            # 16 K-tiles per row
KQ = 4                   # k-tiles per lo quad tile (8B matmul rhs stride)
GR = 8                   # rows per input-DMA group
NLO = 256                # low-digit bins (t & 255)
NHI = 128                # high-digit compare width (t >> 8 < 125)

# Row pairs whose lo one-hots build on ACT (must be {even, even+1} pairs).
ACT_ROWS = frozenset({8, 9, 20, 21, 28, 29})


def _build_nc():
    from contextlib import ExitStack

    from concourse import bacc, bass, mybir
    from concourse.tile import TileContext

    nc = bacc.Bacc()
    docs = nc.dram_tensor("docs", [ROWS, SEQ], mybir.dt.int32, kind="ExternalInput")
    hist = nc.dram_tensor("hist", [ROWS, VOCAB], mybir.dt.float32, kind="ExternalOutput")

    f32 = mybir.dt.float32
    bf16 = mybir.dt.bfloat16
    Alu = mybir.AluOpType
    Act = mybir.ActivationFunctionType

    with TileContext(nc) as tc, ExitStack() as ctx:
        const_tp = ctx.enter_context(tc.tile_pool(name="const", bufs=1))
        tok_tp = ctx.enter_context(tc.tile_pool(name="tok", bufs=4))
        hilo_tp = ctx.enter_context(tc.tile_pool(name="hilo", bufs=4))
        ohh_tp = ctx.enter_context(tc.tile_pool(name="ohh", bufs=5))
        ohl_tp = ctx.enter_context(tc.tile_pool(name="ohl", bufs=3))
        act_tp = ctx.enter_context(tc.tile_pool(name="act", bufs=2))
        res_tp = ctx.enter_context(tc.tile_pool(name="res", bufs=4))
        psum_tp = ctx.enter_context(tc.tile_pool(name="psum", bufs=8, space="PSUM"))

        # Replicated iota constants, k-tile axis innermost (stride 1) so the
        # one-hot tensor_tensor ops keep contiguous last dims (2x DVE mode).
        iota_hi = const_tp.tile([P, NHI, KT], bf16)
        nc.gpsimd.iota(iota_hi[:], [[1, NHI], [0, KT]], channel_multiplier=0,
                       allow_small_or_imprecise_dtypes=True)
        iota_lo = const_tp.tile([P, NLO, KQ], bf16)
        nc.gpsimd.iota(iota_lo[:], [[1, NLO], [0, KQ]], channel_multiplier=0,
                       allow_small_or_imprecise_dtypes=True)
        iota_lof = const_tp.tile([P, NLO], bf16)  # flat, for the ACT path
        nc.gpsimd.iota(iota_lof[:], [[1, NLO]], channel_multiplier=0,
                       allow_small_or_imprecise_dtypes=True)

        # Upfront: all group DMAs + int preprocessing, so the one-hot DVE
        # stream never waits on input loads.
        hi_bfs, lo_bfs, nlo_fs = [], [], []
        for g in range(ROWS // GR):
            tok = tok_tp.tile([P, GR, KT], mybir.dt.int32, tag=f"t{g}")
            src = bass.AP(docs, g * GR * SEQ, [[16, P], [SEQ, GR], [1, KT]])
            nc.sync.dma_start(out=tok[:], in_=src)

            hi_i = hilo_tp.tile([P, GR, KT], mybir.dt.int32, tag=f"hii{g}")
            lo_i = hilo_tp.tile([P, GR, KT], mybir.dt.int32, tag=f"loi{g}")
            nc.vector.tensor_scalar(out=hi_i[:], in0=tok[:], scalar1=8,
                                    scalar2=None, op0=Alu.logical_shift_right)
            nc.vector.tensor_scalar(out=lo_i[:], in0=tok[:], scalar1=255,
                                    scalar2=None, op0=Alu.bitwise_and)
            hi_bf = hilo_tp.tile([P, GR, KT], bf16, tag=f"hib{g}")
            lo_bf = hilo_tp.tile([P, GR, KT], bf16, tag=f"lob{g}")
            nc.vector.tensor_scalar(out=hi_bf[:], in0=hi_i[:], scalar1=1.0,
                                    scalar2=None, op0=Alu.mult)
            nc.vector.tensor_scalar(out=lo_bf[:], in0=lo_i[:], scalar1=1.0,
                                    scalar2=None, op0=Alu.mult)
            nlo_f = None
            if any(g * GR <= r < (g + 1) * GR for r in ACT_ROWS):
                nlo_f = hilo_tp.tile([P, GR, KT], f32, tag=f"nlo{g}")
                nc.vector.tensor_scalar(out=nlo_f[:], in0=lo_i[:],
                                        scalar1=-1.0, scalar2=None,
                                        op0=Alu.mult)
            hi_bfs.append(hi_bf)
            lo_bfs.append(lo_bf)
            nlo_fs.append(nlo_f)

        def emit_matmuls(r_pair, oh_hi, lo_tiles):
            for rr in range(2):
                r = r_pair + rr
                ps = psum_tp.tile([P, NLO], f32)
                for k in range(KT):
                    ent = lo_tiles[(rr, k)]
                    rhs = ent[0][:, ent[1], :, ent[2]] \
                        if isinstance(ent, tuple) else ent[:]
                    nc.tensor.matmul(out=ps[:], lhsT=oh_hi[:, rr, :, k],
                                     rhs=rhs,
                                     start=(k == 0), stop=(k == KT - 1))

                res = res_tp.tile([P, NLO], f32)
                nc.scalar.mul(out=res[:], in_=ps[:], mul=1.0 / SEQ)
                nc.sync.dma_start(
                    out=hist[r].rearrange("(h l) -> h l", l=NLO),
                    in_=res[:VOCAB // NLO, :])

        # ACT-pair matmuls are deferred two pair-slots so the PE keeps
        # streaming DVE pairs while the (slow) ACT builds accumulate;
        # otherwise the PE stalls in issue order, the one-hot rings fill,
        # and the bottleneck DVE engine starves (idle + wake-up penalty).
        pending = []
        for g in range(ROWS // GR):
            hi_bf, lo_bf, nlo_f = hi_bfs[g], lo_bfs[g], nlo_fs[g]
            for r0 in range(0, GR, 2):
                r_pair = g * GR + r0
                on_act = r_pair in ACT_ROWS
                # hi one-hots for 2 rows x 16 k-tiles in one op
                oh_hi = ohh_tp.tile([P, 2, NHI, KT], bf16)
                nc.vector.tensor_tensor(
                    out=oh_hi[:], in0=iota_hi[:].unsqueeze(1)
                        .broadcast_to([P, 2, NHI, KT]),
                    in1=hi_bf[:, r0:r0 + 2, :].unsqueeze(2)
                        .broadcast_to([P, 2, NHI, KT]),
                    op=Alu.is_equal)

                lo_tiles = {}
                if on_act:
                    # ACT path: per (row, k-tile), relu(1 - (iota - lo)^2)
                    for rr in range(2):
                        for k in range(KT):
                            sq = act_tp.tile([P, NLO], bf16, tag=f"sq{rr}")
                            t_lo = act_tp.tile([P, NLO], bf16,
                                               tag=f"a{rr}_{k}")
                            nc.scalar.activation(
                                out=sq[:], in_=iota_lof[:], func=Act.Square,
                                bias=nlo_f[:, r0 + rr, k:k + 1], scale=1.0)
                            nc.scalar.activation(
                                out=t_lo[:], in_=sq[:], func=Act.Relu,
                                bias=1.0, scale=-1.0)
                            lo_tiles[(rr, k)] = t_lo
                    pending.append((r_pair, oh_hi, lo_tiles))
                else:
                    # DVE path: 2-row quad-k tiles [p, 2, bin, 4]
                    for q in range(KT // KQ):
                        t_lo = ohl_tp.tile([P, 2, NLO, KQ], bf16, tag=f"q{q}")
                        nc.vector.tensor_tensor(
                            out=t_lo[:], in0=iota_lo[:].unsqueeze(1)
                                .broadcast_to([P, 2, NLO, KQ]),
                            in1=lo_bf[:, r0:r0 + 2, q * KQ:(q + 1) * KQ]
                                .unsqueeze(2).broadcast_to([P, 2, NLO, KQ]),
                            op=Alu.is_equal)
                        for rr in range(2):
                            for k in range(KQ):
                                lo_tiles[(rr, q * KQ + k)] = (t_lo, rr, k)

                    while pending and pending[0][0] <= r_pair - 4:
                        emit_matmuls(*pending.pop(0))
                    emit_matmuls(r_pair, oh_hi, lo_tiles)
        for args in pending:
            emit_matmuls(*args)
    nc.compile()
    return nc


_NC_CACHE = None


def _get_nc():
    global _NC_CACHE
    if _NC_CACHE is None:
        _NC_CACHE = _build_nc()
    return _NC_CACHE


def run_sharded(docs: np.ndarray, trace: bool = False):
    """Run the 8-core SPMD kernel. Returns (full_output, BassKernelResults)."""
    from concourse.bass_utils import run_bass_kernel_spmd

    docs = np.ascontiguousarray(np.asarray(docs, dtype=np.int32))
    assert docs.shape == (BATCH, SEQ), docs.shape
    shards = docs.reshape(N_CORES, ROWS, SEQ)
    in_maps = [{"docs": shards[i]} for i in range(N_CORES)]
    res = run_bass_kernel_spmd(_get_nc(), in_maps, core_ids=list(range(N_CORES)),
                               trace=trace)
    out = np.concatenate([res.results[i]["hist"] for i in range(N_CORES)], axis=0)
    return out, res


def kernel(docs: np.ndarray) -> np.ndarray:
    out, _ = run_sharded(docs, trace=False)
    return out
